# revision 42
# baseline (speedup 1.0000x reference)
"""Trainium2 Bass kernel for EnhancedTripletLoss — v12 (~96µs, from 117µs).

Scheme (class-sharded SPMD over 8 cores, core c owns the anchors of
label class c):
  * Columns = the first WMAIN embeddings of each class, class-sorted
    into 8 blocks. Class excess beyond WMAIN (a few tens of columns)
    and anchors beyond MC per class are merged in on the host — the
    device-side overflow path cost more in pipeline stalls than the
    work is worth.
  * Per anchor tile, per block: 2 bf16 matmul passes (K=128 halves of
    D=256) accumulate -2*e_i·e_j into PSUM.
  * Direct blocks: custom DVE op TT_ADD_MINRED (body=Src0+Src1,
    accum=min) adds the per-column ||e_j||^2 (fp16, offset by -256) and
    block-min-reduces straight from PSUM at 1 elem/cycle; the body
    output is aliased onto the PSUM tile itself (no SBUF write stream).
  * Y_OFF offload blocks: the sq channel rides the PE (K=3 bf16 terms),
    ACT converts fp32 PSUM -> fp16 SBUF, TT_MIN_MINRED min-reduces the
    block by feeding its halves as Src0/Src1. Balances DVE/ACT/PE.
  * Own-class columns are sign-flipped on the host so block "min" is
    -(block max). Per-(tile,block) mins land directly in a resident
    bmall[P, Mt, NB]; one deferred add + 3D tensor_reduce(min) pair
    produces per-anchor pmins/nmins, which are DMA'd out [P, 2*Mt].
    sqrt/relu/margin/mean run on the host.
  * Startup: 6 full-array warmup matmuls (K=128 — a partial-array
    warmup leaves the chip at 5/6 clock for the WHOLE run, +14%), u
    tiles loaded fully upfront, per-block v DMAs split across the
    sync/gpsimd queues, sqct broadcast trimmed to the direct-block
    region, PSUM pool = 4 bufs (the full 16KB/partition).
  * Drain: tiles >= 2 process direct blocks first and offload blocks
    last; the last tile's final block runs direct (its sqct region
    streams in last), so the kernel drains through one short DVE op
    instead of a 2-op backlog + ACT-copy chain (~-2.5us tail).
"""

import numpy as np
import ml_dtypes

P = 128
D = 256
NCLS = 8
NCORES = 8
MARGIN = 0.3
PD_EPS = 1e-6
BIGM = 1.0e30
SQTERMS = 3
WMAIN = 1024
MT = 8              # fixed: 8 anchor tiles of 128 per core
MC = MT * P
SQOFF = 256.0       # offset folded out of sq for fp16 precision
PADV = 60000.0      # fp16 pad for offload fold tree
Y_OFF = 3           # blocks routed through ACT+pair-min offload

BF16 = ml_dtypes.bfloat16
F8E4 = ml_dtypes.float8_e4m3fn


def _ref_add_minred(in0, in1, c0, c1, c2):
    b = in0.astype(np.float32) + in1.astype(np.float32)
    b2 = b.reshape(b.shape[0], -1)
    seed = np.asarray(c0, np.float32).reshape(-1, 1)
    acc = np.minimum(b2.min(axis=-1, keepdims=True), seed)
    return b, acc


def _ref_min_minred(in0, in1, c0, c1, c2):
    b = np.minimum(in0.astype(np.float32), in1.astype(np.float32))
    b2 = b.reshape(b.shape[0], -1)
    seed = np.asarray(c0, np.float32).reshape(-1, 1)
    acc = np.minimum(b2.min(axis=-1, keepdims=True), seed)
    return b, acc


def _register_op(name, spec):
    from concourse.dve_ops import DveOp, OPS, CUSTOM_DVE_SPECS, _SUB_OPCODE_FOR_NAME

    if name in _SUB_OPCODE_FOR_NAME:
        return next(op for op in OPS if op.name == name)
    op = DveOp(name, spec, subdim=False, uops_sha={})
    row = max(_SUB_OPCODE_FOR_NAME.values()) + 1
    assert row < 0x20
    OPS.append(op)
    CUSTOM_DVE_SPECS[name] = op.spec
    _SUB_OPCODE_FOR_NAME[name] = row
    for ver in ("v3", "v4"):
        try:
            op.compile(ver)
        except ValueError as e:
            import re
            m = re.search(r'="([0-9a-f]{16})"', str(e))
            assert m, f"could not parse sha from: {e}"
            op.uops_sha[ver] = m.group(1)
    op.compile("v3")
    return op


def _ensure_custom_ops():
    from concourse.dve_spec import Spec, Src0, Src1, C0, minn

    add_op = _register_op(
        "TT_ADD_MINRED",
        Spec(body=Src0 + Src1, accum=minn, accum_init=C0,
             reference=_ref_add_minred))
    min_op = _register_op(
        "TT_MIN_MINRED",
        Spec(body=minn(Src0, Src1), accum=minn, accum_init=C0,
             reference=_ref_min_minred))
    return add_op, min_op


def _build_program(wmain, y_off=Y_OFF):
    import concourse.tile as tile
    from concourse import bacc, mybir

    cop, mop = _ensure_custom_ops()

    f32 = mybir.dt.float32
    f16 = mybir.dt.float16
    bf16 = mybir.dt.bfloat16
    f8e4 = mybir.dt.float8e4
    DR = mybir.MatmulPerfMode.DoubleRow
    AX = mybir.AxisListType.X
    OP = mybir.AluOpType

    Mt = MT
    Mc = MC
    NB = NCLS
    N = int(sum(wmain))
    moffs = np.concatenate([[0], np.cumsum(wmain)]).astype(int)
    off_blocks = [b for b in range(y_off) if wmain[b] > 512]
    direct_blocks = [b for b in range(NCLS) if b not in off_blocks]
    dstart = min([int(moffs[b]) for b in direct_blocks] or [N])

    nc = bacc.Bacc("TRN2", target_bir_lowering=False, debug=False)

    vbd = [nc.dram_tensor(f"vb{b}", [P, 2 * int(wmain[b])], bf16,
                          kind="ExternalInput") for b in range(NCLS)]
    v2d = {b: nc.dram_tensor(f"v2b{b}", [2, 2 * int(wmain[b])], f8e4,
                             kind="ExternalInput") for b in off_blocks}
    uts = [
        nc.dram_tensor(f"u{k}t0", [P, Mc], bf16, kind="ExternalInput")
        for k in range(2)
    ]
    sqbd = nc.dram_tensor("sqb16", [1, N], f16, kind="ExternalInput")
    onesd = nc.dram_tensor("ones8", [2, 2 * P], f8e4, kind="ExternalInput")
    pbig = nc.dram_tensor("posbig", [P, Mt * NB], f32, kind="ExternalInput")
    nbig = nc.dram_tensor("negbig", [P, Mt * NB], f32, kind="ExternalInput")
    out = nc.dram_tensor("out", [P, 2 * Mt], f32, kind="ExternalOutput")

    with tile.TileContext(nc) as tc:
        with (
            tc.tile_pool(name="resident", bufs=1) as res,
            tc.tile_pool(name="psum", bufs=4, space="PSUM") as pp,
            tc.tile_pool(name="scrd", bufs=2) as scrp,
            tc.tile_pool(name="epi", bufs=4) as epi,
        ):
            # ones rows for the offload sq channel (K=SQTERMS matmul lhsT);
            # memset first on the idle vector queue — it doubles as the PE
            # warmup source, so the warmup has no DMA-queue dependency.
            u2t = res.tile([P, Mc], bf16, tag="u2")
            nc.vector.memset(u2t[:], 1.0)
            ones8 = res.tile([2, 2, P], f8e4, tag="ones8")
            nc.sync.dma_start(out=ones8[:], in_=onesd[:, :])

            # ---- input DMAs: first-tile operands first --------------------
            dma_engs = [nc.sync, nc.gpsimd]
            _dma_rr = [0]

            def dma(out_ap, in_ap):
                dma_engs[_dma_rr[0] % len(dma_engs)].dma_start(out=out_ap, in_=in_ap)
                _dma_rr[0] += 1

            # u tiles: tile-0 slice first so the first matmul can start as
            # soon as vb0 lands; the rest streams behind the v blocks.
            utiles = []
            for i, ut in enumerate(uts):
                t = res.tile([P, Mc], bf16, tag=f"ut{i}", name=f"ut{i}")
                dma(t[:], ut[:, :])
                utiles.append(t)

            sqct = res.tile([P, N], f16, tag="sqc16")
            v0ts, v1ts, v2ts = [None] * NCLS, [None] * NCLS, {}
            for b in range(NCLS):
                W = int(wmain[b])
                tb = res.tile([P, 2 * W], bf16, tag=f"vb{b}", name=f"vb{b}")
                nc.sync.dma_start(out=tb[:, 0:W], in_=vbd[b][:, 0:W])
                nc.gpsimd.dma_start(out=tb[:, W:2 * W], in_=vbd[b][:, W:2 * W])
                v0ts[b] = tb[:, 0:W]
                v1ts[b] = tb[:, W:2 * W]
                if b in off_blocks:
                    t2 = res.tile([2, 2, W], f8e4, tag=f"v2b{b}",
                                  name=f"v2b{b}")
                    dma(t2[:], v2d[b][:, :])
                    v2ts[b] = t2
                if b == 2:
                    # sq row broadcast — only the direct region is ever read
                    nc.sync.dma_start(
                        out=sqct[:, dstart:N],
                        in_=sqbd[0:1, dstart:N].partition_broadcast(P))

            pbigt = res.tile([P, Mt, NB], f32, tag="posbig")
            dma(pbigt[:], pbig[:, :])
            nbigt = res.tile([P, Mt, NB], f32, tag="negbig")
            dma(nbigt[:], nbig[:, :])

            # sq cols of the last offload block — needed only by the last
            # tile's direct-drain op, so this streams after everything else
            if off_blocks:
                lb = off_blocks[-1]
                nc.gpsimd.dma_start(
                    out=sqct[:, int(moffs[lb]):int(moffs[lb + 1])],
                    in_=sqbd[0:1, int(moffs[lb]):int(moffs[lb + 1])]
                    .partition_broadcast(P))

            # offload conversion tiles (padded to 1024 with fp16 "+inf")
            convs = {}
            for b in off_blocks:
                cb = res.tile([P, WMAIN], f16, tag=f"conv{b}")
                nc.gpsimd.memset(cb[:], PADV)
                convs[b] = cb

            # ---- PE warmup (clock ramp; streams the u2t ones tile) --------
            wp = pp.tile([P, WMAIN], f32, tag="pblk", name="warm")
            for _ in range(6):
                nc.tensor.matmul(wp[:, 0:512], u2t[:, 0:P],
                                 u2t[:, 0:512], start=True, stop=True)

            bmall = res.tile([P, Mt, NB], f32, tag="bmall")
            out_sb = res.tile([P, 2 * Mt], f32, tag="out")

            # ---- main loop ------------------------------------------------
            for mt in range(Mt):
                ms = slice(mt * P, (mt + 1) * P)
                # tiles 0-1 follow DMA arrival order (offload blocks first,
                # direct blocks wait on the late sqct broadcast anyway);
                # later tiles put direct blocks first so the tile drains
                # through the short ACT+fold chain, not a DVE backlog.
                order = (list(range(NCLS)) if mt < 2 else
                         direct_blocks + off_blocks)
                for b in order:
                    W = int(wmain[b])
                    c0 = int(moffs[b])
                    ptile = pp.tile([P, WMAIN], f32, tag="pblk", name="pblk")
                    segs = [(i, min(512, W - i)) for i in range(0, W, 512)]
                    stats = [(utiles[0], v0ts[b]), (utiles[1], v1ts[b])]
                    # last tile ends on a direct op: one 1.1us DVE op
                    # drains faster than the serial ACT-copy + fold chain
                    is_off = (b in off_blocks
                              and not (mt == Mt - 1 and b == off_blocks[-1]))
                    for ti, (ut, vt) in enumerate(stats):
                        last_stat = (ti == len(stats) - 1) and not is_off
                        for i, s in segs:
                            cs = slice(i, i + s)
                            nc.tensor.matmul(
                                ptile[:, cs], ut[:, ms], vt[:, cs],
                                start=(ti == 0), stop=last_stat,
                            )
                    if is_off:
                        # sq channel: 3 fp8 terms as a K=4 DoubleRow matmul
                        # (2 row-pairs, 0.5 cyc/col — half the bf16 channel)
                        for si, (i, s) in enumerate(segs):
                            cs = slice(i, i + s)
                            nc.tensor.matmul(
                                ptile[:, cs],
                                ones8[:, :, :],
                                v2ts[b][:, :, cs],
                                start=False, stop=True,
                                perf_mode=DR,
                            )
                        cb = convs[b]
                        nc.scalar.copy(cb[:, 0:W], ptile[:, 0:W])
                        # fused pair-min reduce: both halves in one 512-pass
                        fsc = scrp.tile([P, 512], f16, tag="fold")
                        nc.vector._custom_dve(
                            mop, out=fsc[:, 0:512], in0=cb[:, 0:512],
                            in1=cb[:, 512:1024], s0=BIGM,
                            accum_out=bmall[:, mt, b:b + 1])
                    else:
                        # fused add+min vs ±(sq-256); body out aliased on PSUM
                        nc.vector._custom_dve(
                            cop, out=ptile[:, 0:W], in0=ptile[:, 0:W],
                            in1=sqct[:, c0:c0 + W], s0=BIGM,
                            accum_out=bmall[:, mt, b:b + 1])

            # ---- deferred epilogue: per-anchor pos/neg mins, DMA out -----
            tp = epi.tile([P, Mt, NB], f32, tag="tp")
            nc.vector.tensor_tensor(tp[:], bmall[:], pbigt[:], op=OP.add)
            nc.vector.tensor_reduce(out_sb[:, 0:Mt], tp[:], axis=AX, op=OP.min)
            tn = epi.tile([P, Mt, NB], f32, tag="tn")
            nc.vector.tensor_tensor(tn[:], bmall[:], nbigt[:], op=OP.add)
            nc.vector.tensor_reduce(out_sb[:, Mt:2 * Mt], tn[:], axis=AX,
                                    op=OP.min)
            nc.sync.dma_start(out=out[:, :], in_=out_sb[:])

    nc.compile()
    return nc


def _prepare_inputs(emb, lab):
    B = emb.shape[0]
    assert emb.shape[1] == D
    counts = np.bincount(lab, minlength=NCLS).astype(int)
    assert counts.sum() == B

    order = np.argsort(lab, kind="stable")
    cstart = np.concatenate([[0], np.cumsum(counts)]).astype(int)

    wmain = tuple(max(1, min(int(n), WMAIN)) for n in counts)
    NB = NCLS
    Mt = MT
    Mc = MC
    N = int(sum(wmain))
    off_blocks = [b for b in range(Y_OFF) if wmain[b] > 512]

    sq = np.einsum("ij,ij->i", emb, emb, dtype=np.float32)

    colidx = np.empty(N, dtype=np.int64)
    own_ranges = {}
    extra_cols = []          # class excess beyond WMAIN -> host merge
    off = 0
    for c in range(NCLS):
        idx = order[cstart[c]:cstart[c + 1]][:wmain[c]]
        extra_cols.extend(order[cstart[c] + wmain[c]:cstart[c + 1]].tolist())
        if len(idx) == 0:
            idx = order[0:1]
        w = wmain[c]
        colidx[off:off + w] = idx
        own_ranges[c] = (off, w)
        off += w

    Vg = np.ascontiguousarray(emb[colidx].T).astype(BF16)     # [256, N]
    sqo = sq - np.float32(SQOFF)                              # sq - 256
    sq_terms = []
    r = sqo.astype(np.float32)
    for _ in range(SQTERMS):
        h = r.astype(F8E4)
        sq_terms.append(h)
        r = r - h.astype(np.float32)
    sqf_t = np.stack([t[colidx] for t in sq_terms])           # [SQTERMS, N]

    u_full = (-2.0 * emb).astype(BF16)

    # anchors beyond MC per class are handled on the host
    host_anchors = []
    anchor_map = []          # per core: global anchor index per [P, Mt] slot
    in_maps = []
    for c in range(NCLS):
        aidx_all = order[cstart[c]:cstart[c + 1]]
        aidx = aidx_all[:Mc]
        host_anchors.extend(aidx_all[Mc:].tolist())
        if len(aidx) == 0:
            aidx = order[0:1]
        npad = Mc - len(aidx)
        pad = np.full(npad, aidx[0], dtype=np.int64)
        aidx_p = np.concatenate([aidx, pad])
        nreal = len(aidx)

        s = np.ones(N, dtype=np.float32)
        o, w = own_ranges[c]
        s[o:o + w] = -1.0
        sb = s.astype(BF16)

        posbig = np.zeros((P, NB), dtype=np.float32)
        negbig = np.zeros((P, NB), dtype=np.float32)
        for j in range(NCLS):
            if j == c:
                negbig[:, j] = BIGM
            else:
                posbig[:, j] = BIGM

        vv0 = Vg[0:128] * sb
        vv1 = Vg[128:256] * sb
        vv2 = (sqf_t.astype(np.float32) * s).astype(F8E4)
        sqb16 = (sqo[colidx] * s).astype(np.float16).reshape(1, N)
        o8 = np.ones((2, 2, P), dtype=F8E4)
        o8[1, 1, :] = 0.0
        im = {
            "posbig": np.ascontiguousarray(np.tile(posbig, (1, Mt))),
            "negbig": np.ascontiguousarray(np.tile(negbig, (1, Mt))),
            "sqb16": sqb16,
            "ones8": np.ascontiguousarray(o8.reshape(2, 2 * P)),
        }
        off2 = 0
        for b in range(NCLS):
            w = wmain[b]
            im[f"vb{b}"] = np.ascontiguousarray(np.concatenate(
                [vv0[:, off2:off2 + w], vv1[:, off2:off2 + w]], axis=1))
            if b in off_blocks:
                pk = np.zeros((2, 2, w), dtype=F8E4)
                pk[0, 0] = vv2[0, off2:off2 + w]
                pk[0, 1] = vv2[1, off2:off2 + w]
                pk[1, 0] = vv2[2, off2:off2 + w]
                im[f"v2b{b}"] = np.ascontiguousarray(pk.reshape(2, 2 * w))
            off2 += w
        ut = u_full[aidx_p]
        im["u0t0"] = np.ascontiguousarray(ut[:, 0:128].T)
        im["u1t0"] = np.ascontiguousarray(ut[:, 128:256].T)
        in_maps.append(im)
        anchor_map.append((aidx_p.copy(), nreal))

    meta = dict(wmain=wmain, host_anchors=host_anchors,
                extra_cols=extra_cols, anchor_map=anchor_map, counts=counts)
    return in_maps, meta


def _host_terms(emb, lab, sq, host_anchors):
    """num/den contributions of anchors handled fully on the host."""
    if not host_anchors:
        return 0.0, 0.0
    ai = np.asarray(host_anchors, dtype=np.int64)
    d2 = sq[ai][:, None] + sq[None, :] - 2.0 * (emb[ai] @ emb.T)
    Dm = np.sqrt(np.maximum(d2, 0.0))
    eq = lab[ai][:, None] == lab[None, :]
    pos = eq.copy()
    pos[np.arange(len(ai)), ai] = False
    neg = ~eq
    pos_idx = np.argmax(np.where(pos, Dm, -np.inf), axis=1)
    neg_idx = np.argmin(np.where(neg, Dm, np.inf), axis=1)
    valid = pos.any(axis=1) & neg.any(axis=1)
    pos_dist = np.linalg.norm(emb[ai] - emb[pos_idx] + PD_EPS, axis=1)
    neg_dist = np.linalg.norm(emb[ai] - emb[neg_idx] + PD_EPS, axis=1)
    per = np.maximum(pos_dist - neg_dist + MARGIN, 0.0)
    num = float(np.sum(np.where(valid, per, 0.0)))
    den = float(valid.sum())
    return num, den


_PROGRAM_CACHE = {}


def _get_program(wmain):
    key = (wmain, Y_OFF, "v14")
    if key not in _PROGRAM_CACHE:
        _PROGRAM_CACHE[key] = _build_program(wmain)
    return _PROGRAM_CACHE[key]


def _combine(results, meta, emb, lab, sq):
    """Merge device per-anchor mins with host-side extra cols/anchors."""
    B = emb.shape[0]
    counts = meta["counts"]
    extra_cols = np.asarray(meta["extra_cols"], dtype=np.int64)

    num = 0.0
    den = 0.0
    for c in range(NCLS):
        aidx_p, nreal = meta["anchor_map"][c]
        if counts[c] < 2 or counts[c] > B - 1 or nreal == 0:
            continue
        o = np.asarray(results[c]["out"], dtype=np.float32)  # [P, 2*Mt]
        pmins = o[:, 0:MT].T.reshape(-1)[:nreal]             # [nreal]
        nmins = o[:, MT:2 * MT].T.reshape(-1)[:nreal]
        ai = aidx_p[:nreal]
        sqa = sq[ai] + np.float32(SQOFF)
        pos_d2 = np.maximum(sqa - pmins, 0.0)
        neg_d2 = np.maximum(sqa + nmins, 0.0)

        if len(extra_cols):
            d2x = (sq[ai][:, None] + sq[extra_cols][None, :]
                   - 2.0 * (emb[ai] @ emb[extra_cols].T))    # [nreal, nx]
            d2x = np.maximum(d2x, 0.0)
            same = lab[extra_cols][None, :] == c
            posx = np.where(same, d2x, -np.inf).max(axis=1)
            negx = np.where(~same, d2x, np.inf).min(axis=1)
            pos_d2 = np.maximum(pos_d2, posx)
            neg_d2 = np.minimum(neg_d2, negx)

        per = np.maximum(np.sqrt(pos_d2) - np.sqrt(neg_d2) + MARGIN, 0.0)
        num += float(per.sum())
        den += float(nreal)

    hnum, hden = _host_terms(emb, lab, sq, meta["host_anchors"])
    num += hnum
    den += hden
    return np.float32(num / max(den, 1.0))


def _setup_trace_hook():
    import sys
    import types
    try:
        from antenv.axon_hooks import get_axon_ntff_profile_hook  # noqa: F401
        return
    except ImportError:
        pass
    import antenv
    from trn_agent_boot.trn_boot import _ntff_profile_via_ctypes

    mod = types.ModuleType("antenv.axon_hooks")
    state = {"h": None}
    mod.set_axon_ntff_profile_hook = lambda h: state.__setitem__("h", h)
    mod.get_axon_ntff_profile_hook = lambda: state["h"]
    sys.modules["antenv.axon_hooks"] = mod
    antenv.axon_hooks = mod
    mod.set_axon_ntff_profile_hook(
        _ntff_profile_via_ctypes("/opt/axon/libaxon_pjrt.so")
    )


def kernel(embeddings, labels, _trace=False):
    emb = np.ascontiguousarray(np.asarray(embeddings, dtype=np.float32))
    lab = np.asarray(labels).astype(np.int64).ravel()

    in_maps, meta = _prepare_inputs(emb, lab)
    nc = _get_program(meta["wmain"])

    sq = np.einsum("ij,ij->i", emb, emb, dtype=np.float32)

    from concourse.bass_utils import run_bass_kernel_spmd

    if _trace:
        _setup_trace_hook()
        import concourse.bass_utils as _bu
        _bu.upload_artifacts = lambda tmpdir: tmpdir

    res = run_bass_kernel_spmd(
        nc, in_maps, core_ids=list(range(NCORES)), trace=bool(_trace),
    )
    loss = _combine(res.results, meta, emb, lab, sq)
    if _trace:
        return loss, res
    return loss


# revision 43
# speedup vs baseline: 1.0821x; 1.0821x over previous
"""Trainium2 Bass kernel for EnhancedTripletLoss — v12 (~96µs, from 117µs).

Scheme (class-sharded SPMD over 8 cores, core c owns the anchors of
label class c):
  * Columns = the first WMAIN embeddings of each class, class-sorted
    into 8 blocks. Class excess beyond WMAIN (a few tens of columns)
    and anchors beyond MC per class are merged in on the host — the
    device-side overflow path cost more in pipeline stalls than the
    work is worth.
  * Per anchor tile, per block: 2 bf16 matmul passes (K=128 halves of
    D=256) accumulate -2*e_i·e_j into PSUM.
  * Direct blocks: custom DVE op TT_ADD_MINRED (body=Src0+Src1,
    accum=min) adds the per-column ||e_j||^2 (fp16, offset by -256) and
    block-min-reduces straight from PSUM at 1 elem/cycle; the body
    output is aliased onto the PSUM tile itself (no SBUF write stream).
  * Y_OFF offload blocks: the sq channel rides the PE (K=3 bf16 terms),
    ACT converts fp32 PSUM -> fp16 SBUF, TT_MIN_MINRED min-reduces the
    block by feeding its halves as Src0/Src1. Balances DVE/ACT/PE.
  * Own-class columns are sign-flipped on the host so block "min" is
    -(block max). Per-(tile,block) mins land directly in a resident
    bmall[P, Mt, NB]; one deferred add + 3D tensor_reduce(min) pair
    produces per-anchor pmins/nmins, which are DMA'd out [P, 2*Mt].
    sqrt/relu/margin/mean run on the host.
  * Startup: 6 full-array warmup matmuls (K=128 — a partial-array
    warmup leaves the chip at 5/6 clock for the WHOLE run, +14%), u
    tiles loaded fully upfront, per-block v DMAs split across the
    sync/gpsimd queues, sqct broadcast trimmed to the direct-block
    region, PSUM pool = 4 bufs (the full 16KB/partition).
  * Drain: tiles >= 2 process direct blocks first and offload blocks
    last; the last tile's final block runs direct (its sqct region
    streams in last), so the kernel drains through one short DVE op
    instead of a 2-op backlog + ACT-copy chain (~-2.5us tail).
"""

import numpy as np
import ml_dtypes

P = 128
D = 256
NCLS = 8
NCORES = 8
MARGIN = 0.3
PD_EPS = 1e-6
BIGM = 1.0e30
SQTERMS = 3
WMAIN = 1024
MT = 8              # fixed: 8 anchor tiles of 128 per core
MC = MT * P
SQOFF = 256.0       # offset folded out of sq for fp16 precision
PADV = 60000.0      # fp16 pad for offload fold tree
Y_OFF = 3           # blocks routed through ACT+pair-min offload

BF16 = ml_dtypes.bfloat16


def _ref_add_minred(in0, in1, c0, c1, c2):
    b = in0.astype(np.float32) + in1.astype(np.float32)
    b2 = b.reshape(b.shape[0], -1)
    seed = np.asarray(c0, np.float32).reshape(-1, 1)
    acc = np.minimum(b2.min(axis=-1, keepdims=True), seed)
    return b, acc


def _ref_min_minred(in0, in1, c0, c1, c2):
    b = np.minimum(in0.astype(np.float32), in1.astype(np.float32))
    b2 = b.reshape(b.shape[0], -1)
    seed = np.asarray(c0, np.float32).reshape(-1, 1)
    acc = np.minimum(b2.min(axis=-1, keepdims=True), seed)
    return b, acc


def _register_op(name, spec):
    from concourse.dve_ops import DveOp, OPS, CUSTOM_DVE_SPECS, _SUB_OPCODE_FOR_NAME

    if name in _SUB_OPCODE_FOR_NAME:
        return next(op for op in OPS if op.name == name)
    op = DveOp(name, spec, subdim=False, uops_sha={})
    row = max(_SUB_OPCODE_FOR_NAME.values()) + 1
    assert row < 0x20
    OPS.append(op)
    CUSTOM_DVE_SPECS[name] = op.spec
    _SUB_OPCODE_FOR_NAME[name] = row
    for ver in ("v3", "v4"):
        try:
            op.compile(ver)
        except ValueError as e:
            import re
            m = re.search(r'="([0-9a-f]{16})"', str(e))
            assert m, f"could not parse sha from: {e}"
            op.uops_sha[ver] = m.group(1)
    op.compile("v3")
    return op


def _ensure_custom_ops():
    from concourse.dve_spec import Spec, Src0, Src1, C0, minn

    add_op = _register_op(
        "TT_ADD_MINRED",
        Spec(body=Src0 + Src1, accum=minn, accum_init=C0,
             reference=_ref_add_minred))
    min_op = _register_op(
        "TT_MIN_MINRED",
        Spec(body=minn(Src0, Src1), accum=minn, accum_init=C0,
             reference=_ref_min_minred))
    return add_op, min_op


def _build_program(wmain, y_off=Y_OFF):
    import concourse.tile as tile
    from concourse import bacc, mybir

    cop, mop = _ensure_custom_ops()

    f32 = mybir.dt.float32
    f16 = mybir.dt.float16
    bf16 = mybir.dt.bfloat16
    AX = mybir.AxisListType.X
    OP = mybir.AluOpType

    Mt = MT
    Mc = MC
    NB = NCLS
    N = int(sum(wmain))
    moffs = np.concatenate([[0], np.cumsum(wmain)]).astype(int)
    off_blocks = [b for b in range(y_off) if wmain[b] > 512]
    direct_blocks = [b for b in range(NCLS) if b not in off_blocks]
    dstart = min([int(moffs[b]) for b in direct_blocks] or [N])

    nc = bacc.Bacc("TRN2", target_bir_lowering=False, debug=False)

    vbd = [nc.dram_tensor(f"vb{b}", [P, 2 * int(wmain[b])], bf16,
                          kind="ExternalInput") for b in range(NCLS)]
    v2d = {b: nc.dram_tensor(f"v2b{b}", [SQTERMS, int(wmain[b])], bf16,
                             kind="ExternalInput") for b in off_blocks}
    uts = [
        nc.dram_tensor(f"u{k}t0", [P, Mc], bf16, kind="ExternalInput")
        for k in range(2)
    ]
    sqbd = nc.dram_tensor("sqb16", [1, N], f16, kind="ExternalInput")
    pbig = nc.dram_tensor("posbig", [P, Mt * NB], f32, kind="ExternalInput")
    nbig = nc.dram_tensor("negbig", [P, Mt * NB], f32, kind="ExternalInput")
    out = nc.dram_tensor("out", [P, 2 * Mt], f32, kind="ExternalOutput")

    with tile.TileContext(nc) as tc:
        with (
            tc.tile_pool(name="resident", bufs=1) as res,
            tc.tile_pool(name="psum", bufs=4, space="PSUM") as pp,
            tc.tile_pool(name="scrd", bufs=2) as scrp,
            tc.tile_pool(name="epi", bufs=4) as epi,
        ):
            # ones rows for the offload sq channel (K=SQTERMS matmul lhsT);
            # memset first on the idle vector queue — it doubles as the PE
            # warmup source, so the warmup has no DMA-queue dependency.
            u2t = res.tile([P, Mc], bf16, tag="u2")
            nc.vector.memset(u2t[:], 1.0)

            # ---- input DMAs: first-tile operands first --------------------
            dma_engs = [nc.sync, nc.gpsimd]
            _dma_rr = [0]

            def dma(out_ap, in_ap):
                dma_engs[_dma_rr[0] % len(dma_engs)].dma_start(out=out_ap, in_=in_ap)
                _dma_rr[0] += 1

            # u tiles: tile-0 slice first so the first matmul can start as
            # soon as vb0 lands; the rest streams behind the v blocks.
            utiles = []
            for i, ut in enumerate(uts):
                t = res.tile([P, Mc], bf16, tag=f"ut{i}", name=f"ut{i}")
                dma(t[:], ut[:, :])
                utiles.append(t)

            sqct = res.tile([P, N], f16, tag="sqc16")
            v0ts, v1ts, v2ts = [None] * NCLS, [None] * NCLS, {}
            for b in range(NCLS):
                W = int(wmain[b])
                tb = res.tile([P, 2 * W], bf16, tag=f"vb{b}", name=f"vb{b}")
                nc.sync.dma_start(out=tb[:, 0:W], in_=vbd[b][:, 0:W])
                nc.gpsimd.dma_start(out=tb[:, W:2 * W], in_=vbd[b][:, W:2 * W])
                v0ts[b] = tb[:, 0:W]
                v1ts[b] = tb[:, W:2 * W]
                if b in off_blocks:
                    t2 = res.tile([32 + SQTERMS, W], bf16, tag=f"v2b{b}",
                                  name=f"v2b{b}")
                    for rp in (0, 32):
                        dma(t2[rp:rp + SQTERMS, :], v2d[b][:, :])
                    v2ts[b] = t2
                if b == 2:
                    # sq row broadcast — only the direct region is ever read
                    nc.sync.dma_start(
                        out=sqct[:, dstart:N],
                        in_=sqbd[0:1, dstart:N].partition_broadcast(P))

            pbigt = res.tile([P, Mt, NB], f32, tag="posbig")
            dma(pbigt[:], pbig[:, :])
            nbigt = res.tile([P, Mt, NB], f32, tag="negbig")
            dma(nbigt[:], nbig[:, :])

            # sq cols of the last offload block — needed only by the last
            # tile's direct-drain op, so this streams after everything else
            if off_blocks:
                lb = off_blocks[-1]
                nc.gpsimd.dma_start(
                    out=sqct[:, int(moffs[lb]):int(moffs[lb + 1])],
                    in_=sqbd[0:1, int(moffs[lb]):int(moffs[lb + 1])]
                    .partition_broadcast(P))

            # offload conversion tiles (padded to 1024 with fp16 "+inf")
            convs = {}
            for b in off_blocks:
                cb = res.tile([P, WMAIN], f16, tag=f"conv{b}")
                nc.gpsimd.memset(cb[:], PADV)
                convs[b] = cb

            # ---- PE warmup (clock ramp; streams the u2t ones tile) --------
            wp = pp.tile([P, WMAIN], f32, tag="pblk", name="warm")
            for _ in range(6):
                nc.tensor.matmul(wp[:, 0:512], u2t[:, 0:P],
                                 u2t[:, 0:512], start=True, stop=True)

            bmall = res.tile([P, Mt, NB], f32, tag="bmall")
            out_sb = res.tile([P, 2 * Mt], f32, tag="out")

            # ---- main loop ------------------------------------------------
            for mt in range(Mt):
                ms = slice(mt * P, (mt + 1) * P)
                # tiles 0-1 follow DMA arrival order (offload blocks first,
                # direct blocks wait on the late sqct broadcast anyway);
                # later tiles put direct blocks first so the tile drains
                # through the short ACT+fold chain, not a DVE backlog.
                order = (list(range(NCLS)) if mt < 2 else
                         direct_blocks + off_blocks)
                for b in order:
                    W = int(wmain[b])
                    c0 = int(moffs[b])
                    ptile = pp.tile([P, WMAIN], f32, tag="pblk", name="pblk")
                    segs = [(i, min(512, W - i)) for i in range(0, W, 512)]
                    stats = [(utiles[0], v0ts[b]), (utiles[1], v1ts[b])]
                    # last tile ends on a direct op: one 1.1us DVE op
                    # drains faster than the serial ACT-copy + fold chain
                    is_off = (b in off_blocks
                              and not (mt == Mt - 1 and b == off_blocks[-1]))
                    for ti, (ut, vt) in enumerate(stats):
                        last_stat = (ti == len(stats) - 1) and not is_off
                        for i, s in segs:
                            cs = slice(i, i + s)
                            nc.tensor.matmul(
                                ptile[:, cs], ut[:, ms], vt[:, cs],
                                start=(ti == 0), stop=last_stat,
                            )
                    if is_off:
                        # K=3 sq channel rides the PE for offload blocks
                        for si, (i, s) in enumerate(segs):
                            cs = slice(i, i + s)
                            rp = 32 * (si % 2)
                            nc.tensor.matmul(
                                ptile[:, cs],
                                u2t[rp:rp + SQTERMS, ms],
                                v2ts[b][rp:rp + SQTERMS, cs],
                                start=False, stop=True,
                                tile_position=(rp, 0),
                            )
                        cb = convs[b]
                        nc.scalar.copy(cb[:, 0:W], ptile[:, 0:W])
                        # fused pair-min reduce: both halves in one 512-pass
                        fsc = scrp.tile([P, 512], f16, tag="fold")
                        nc.vector._custom_dve(
                            mop, out=fsc[:, 0:512], in0=cb[:, 0:512],
                            in1=cb[:, 512:1024], s0=BIGM,
                            accum_out=bmall[:, mt, b:b + 1])
                    else:
                        # fused add+min vs ±(sq-256); body out aliased on PSUM
                        nc.vector._custom_dve(
                            cop, out=ptile[:, 0:W], in0=ptile[:, 0:W],
                            in1=sqct[:, c0:c0 + W], s0=BIGM,
                            accum_out=bmall[:, mt, b:b + 1])

            # ---- deferred epilogue: per-anchor pos/neg mins, DMA out -----
            tp = epi.tile([P, Mt, NB], f32, tag="tp")
            nc.vector.tensor_tensor(tp[:], bmall[:], pbigt[:], op=OP.add)
            nc.vector.tensor_reduce(out_sb[:, 0:Mt], tp[:], axis=AX, op=OP.min)
            tn = epi.tile([P, Mt, NB], f32, tag="tn")
            nc.vector.tensor_tensor(tn[:], bmall[:], nbigt[:], op=OP.add)
            nc.vector.tensor_reduce(out_sb[:, Mt:2 * Mt], tn[:], axis=AX,
                                    op=OP.min)
            nc.sync.dma_start(out=out[:, :], in_=out_sb[:])

    nc.compile()
    return nc


def _prepare_inputs(emb, lab):
    B = emb.shape[0]
    assert emb.shape[1] == D
    counts = np.bincount(lab, minlength=NCLS).astype(int)
    assert counts.sum() == B

    order = np.argsort(lab, kind="stable")
    cstart = np.concatenate([[0], np.cumsum(counts)]).astype(int)

    wmain = tuple(max(1, min(int(n), WMAIN)) for n in counts)
    NB = NCLS
    Mt = MT
    Mc = MC
    N = int(sum(wmain))
    off_blocks = [b for b in range(Y_OFF) if wmain[b] > 512]

    sq = np.einsum("ij,ij->i", emb, emb, dtype=np.float32)

    colidx = np.empty(N, dtype=np.int64)
    own_ranges = {}
    extra_cols = []          # class excess beyond WMAIN -> host merge
    off = 0
    for c in range(NCLS):
        idx = order[cstart[c]:cstart[c + 1]][:wmain[c]]
        extra_cols.extend(order[cstart[c] + wmain[c]:cstart[c + 1]].tolist())
        if len(idx) == 0:
            idx = order[0:1]
        w = wmain[c]
        colidx[off:off + w] = idx
        own_ranges[c] = (off, w)
        off += w

    Vg = np.ascontiguousarray(emb[colidx].T).astype(BF16)     # [256, N]
    sqo = sq - np.float32(SQOFF)                              # sq - 256
    sq_terms = []
    r = sqo.astype(np.float32)
    for _ in range(SQTERMS):
        h = r.astype(BF16)
        sq_terms.append(h)
        r = r - h.astype(np.float32)
    sqf_t = np.stack([t[colidx] for t in sq_terms])           # [SQTERMS, N]

    u_full = (-2.0 * emb).astype(BF16)

    # anchors beyond MC per class are handled on the host
    host_anchors = []
    anchor_map = []          # per core: global anchor index per [P, Mt] slot
    in_maps = []
    for c in range(NCLS):
        aidx_all = order[cstart[c]:cstart[c + 1]]
        aidx = aidx_all[:Mc]
        host_anchors.extend(aidx_all[Mc:].tolist())
        if len(aidx) == 0:
            aidx = order[0:1]
        npad = Mc - len(aidx)
        pad = np.full(npad, aidx[0], dtype=np.int64)
        aidx_p = np.concatenate([aidx, pad])
        nreal = len(aidx)

        s = np.ones(N, dtype=np.float32)
        o, w = own_ranges[c]
        s[o:o + w] = -1.0
        sb = s.astype(BF16)

        posbig = np.zeros((P, NB), dtype=np.float32)
        negbig = np.zeros((P, NB), dtype=np.float32)
        for j in range(NCLS):
            if j == c:
                negbig[:, j] = BIGM
            else:
                posbig[:, j] = BIGM

        vv0 = Vg[0:128] * sb
        vv1 = Vg[128:256] * sb
        vv2 = sqf_t * sb
        sqb16 = (sqo[colidx] * s).astype(np.float16).reshape(1, N)
        im = {
            "posbig": np.ascontiguousarray(np.tile(posbig, (1, Mt))),
            "negbig": np.ascontiguousarray(np.tile(negbig, (1, Mt))),
            "sqb16": sqb16,
        }
        off2 = 0
        for b in range(NCLS):
            w = wmain[b]
            im[f"vb{b}"] = np.ascontiguousarray(np.concatenate(
                [vv0[:, off2:off2 + w], vv1[:, off2:off2 + w]], axis=1))
            if b in off_blocks:
                im[f"v2b{b}"] = np.ascontiguousarray(vv2[:, off2:off2 + w])
            off2 += w
        ut = u_full[aidx_p]
        im["u0t0"] = np.ascontiguousarray(ut[:, 0:128].T)
        im["u1t0"] = np.ascontiguousarray(ut[:, 128:256].T)
        in_maps.append(im)
        anchor_map.append((aidx_p.copy(), nreal))

    meta = dict(wmain=wmain, host_anchors=host_anchors,
                extra_cols=extra_cols, anchor_map=anchor_map, counts=counts)
    return in_maps, meta


def _host_terms(emb, lab, sq, host_anchors):
    """num/den contributions of anchors handled fully on the host."""
    if not host_anchors:
        return 0.0, 0.0
    ai = np.asarray(host_anchors, dtype=np.int64)
    d2 = sq[ai][:, None] + sq[None, :] - 2.0 * (emb[ai] @ emb.T)
    Dm = np.sqrt(np.maximum(d2, 0.0))
    eq = lab[ai][:, None] == lab[None, :]
    pos = eq.copy()
    pos[np.arange(len(ai)), ai] = False
    neg = ~eq
    pos_idx = np.argmax(np.where(pos, Dm, -np.inf), axis=1)
    neg_idx = np.argmin(np.where(neg, Dm, np.inf), axis=1)
    valid = pos.any(axis=1) & neg.any(axis=1)
    pos_dist = np.linalg.norm(emb[ai] - emb[pos_idx] + PD_EPS, axis=1)
    neg_dist = np.linalg.norm(emb[ai] - emb[neg_idx] + PD_EPS, axis=1)
    per = np.maximum(pos_dist - neg_dist + MARGIN, 0.0)
    num = float(np.sum(np.where(valid, per, 0.0)))
    den = float(valid.sum())
    return num, den


_PROGRAM_CACHE = {}


def _get_program(wmain):
    key = (wmain, Y_OFF, "v8")
    if key not in _PROGRAM_CACHE:
        _PROGRAM_CACHE[key] = _build_program(wmain)
    return _PROGRAM_CACHE[key]


def _combine(results, meta, emb, lab, sq):
    """Merge device per-anchor mins with host-side extra cols/anchors."""
    B = emb.shape[0]
    counts = meta["counts"]
    extra_cols = np.asarray(meta["extra_cols"], dtype=np.int64)

    num = 0.0
    den = 0.0
    for c in range(NCLS):
        aidx_p, nreal = meta["anchor_map"][c]
        if counts[c] < 2 or counts[c] > B - 1 or nreal == 0:
            continue
        o = np.asarray(results[c]["out"], dtype=np.float32)  # [P, 2*Mt]
        pmins = o[:, 0:MT].T.reshape(-1)[:nreal]             # [nreal]
        nmins = o[:, MT:2 * MT].T.reshape(-1)[:nreal]
        ai = aidx_p[:nreal]
        sqa = sq[ai] + np.float32(SQOFF)
        pos_d2 = np.maximum(sqa - pmins, 0.0)
        neg_d2 = np.maximum(sqa + nmins, 0.0)

        if len(extra_cols):
            d2x = (sq[ai][:, None] + sq[extra_cols][None, :]
                   - 2.0 * (emb[ai] @ emb[extra_cols].T))    # [nreal, nx]
            d2x = np.maximum(d2x, 0.0)
            same = lab[extra_cols][None, :] == c
            posx = np.where(same, d2x, -np.inf).max(axis=1)
            negx = np.where(~same, d2x, np.inf).min(axis=1)
            pos_d2 = np.maximum(pos_d2, posx)
            neg_d2 = np.minimum(neg_d2, negx)

        per = np.maximum(np.sqrt(pos_d2) - np.sqrt(neg_d2) + MARGIN, 0.0)
        num += float(per.sum())
        den += float(nreal)

    hnum, hden = _host_terms(emb, lab, sq, meta["host_anchors"])
    num += hnum
    den += hden
    return np.float32(num / max(den, 1.0))


def _setup_trace_hook():
    import sys
    import types
    try:
        from antenv.axon_hooks import get_axon_ntff_profile_hook  # noqa: F401
        return
    except ImportError:
        pass
    import antenv
    from trn_agent_boot.trn_boot import _ntff_profile_via_ctypes

    mod = types.ModuleType("antenv.axon_hooks")
    state = {"h": None}
    mod.set_axon_ntff_profile_hook = lambda h: state.__setitem__("h", h)
    mod.get_axon_ntff_profile_hook = lambda: state["h"]
    sys.modules["antenv.axon_hooks"] = mod
    antenv.axon_hooks = mod
    mod.set_axon_ntff_profile_hook(
        _ntff_profile_via_ctypes("/opt/axon/libaxon_pjrt.so")
    )


def kernel(embeddings, labels, _trace=False):
    emb = np.ascontiguousarray(np.asarray(embeddings, dtype=np.float32))
    lab = np.asarray(labels).astype(np.int64).ravel()

    in_maps, meta = _prepare_inputs(emb, lab)
    nc = _get_program(meta["wmain"])

    sq = np.einsum("ij,ij->i", emb, emb, dtype=np.float32)

    from concourse.bass_utils import run_bass_kernel_spmd

    if _trace:
        _setup_trace_hook()
        import concourse.bass_utils as _bu
        _bu.upload_artifacts = lambda tmpdir: tmpdir

    res = run_bass_kernel_spmd(
        nc, in_maps, core_ids=list(range(NCORES)), trace=bool(_trace),
    )
    loss = _combine(res.results, meta, emb, lab, sq)
    if _trace:
        return loss, res
    return loss
